# revision 6
# baseline (speedup 1.0000x reference)
"""DeepseekV2 MoE gate (noaux_tc sigmoid routing) on 8 Trainium2 cores, v2.

Strategy
--------
Token-parallel SPMD: each core routes a 1024-token slice.  All data
marshaling that doesn't need the device happens on the host: x is
transposed, split into bf16 hi/lo, and pre-tiled so every device DMA is
a long contiguous run per partition.

Device program per core (instruction-count-minimized):
  - logits^T = W @ x^T computed with W chunks stationary and 512-token
    moving operands, in plain fp32 (the PE's self-loading fp32 matmult):
    56 k-tiles x 2 expert-halves x 2 token batches = 224 matmuls into
    four [128e, 512t] PSUM accumulators.  fp32 operands make the logits
    exact (no hi/lo split, no selection flips) and halve the PE
    instruction count again: fp32 matmults carry their own weight load.
  - 16 PE transposes restore [token, expert] tiles; ACT applies sigmoid.
  - noaux_tc group-limited top-8 runs as a stage-major chain across the
    8 token tiles with every per-token-independent stage batched into a
    single wide DVE op over all 8 tiles; only the row-global sort ops
    (max8 / max_index / match_replace) remain per-tile.
  - top-k weights are gathered from the unbiased scores with an
    index-match (one-hot vs iota); results stage into two SBUF tiles
    and leave through two DMAs (topk_idx is written as uint32 - the
    host reinterprets, values are 0..255).
"""

import numpy as np

P = 128
TOKENS, HIDDEN, NEXP = 8192, 7168, 256
NCORES = 8
T_CORE = TOKENS // NCORES
TOP_K = 8
N_GROUP = 8
TOPK_GROUP = 4
ROUTED_SCALE = 2.5
NEG_INF = -1.0e9

KT = HIDDEN // P          # 56 contraction k-tiles
NB = 2                    # token batches per core
TB = T_CORE // NB         # 512 tokens per batch
NTT = T_CORE // P         # 8 token tiles per core
CHUNKS = (8, 12, 12, 12, 12)   # k-tiles per x DMA chunk (fast start)
CHUNKS_LOOP = (8,) * 7         # uniform variant for For_i repeat builds


def build_program(repeat=1, legalize=True):
    from contextlib import ExitStack

    import concourse.bass as bass
    import concourse.mybir as mybir
    from concourse.masks import make_identity
    from concourse.tile import TileContext

    f32 = mybir.dt.float32
    bf16 = mybir.dt.bfloat16
    i32 = mybir.dt.int32
    u32 = mybir.dt.uint32
    AO = mybir.AluOpType
    AX = mybir.AxisListType

    nc = bass.Bass()
    xt_d = nc.declare_dram_parameter("xt", [P, NB * KT * TB], f32, isOutput=False)
    wt_d = nc.declare_dram_parameter("wt", [P, KT * NEXP], f32, isOutput=False)
    bias_d = nc.declare_dram_parameter("bias", [NEXP], f32, isOutput=False)
    oi_d = nc.declare_dram_parameter("topk_idx", [T_CORE, TOP_K], u32, isOutput=True)
    ow_d = nc.declare_dram_parameter("topk_w", [T_CORE, TOP_K], f32, isOutput=True)

    with TileContext(nc) as tc, ExitStack() as ctx:
        consts = ctx.enter_context(tc.tile_pool(name="consts", bufs=1))
        wpool = ctx.enter_context(tc.tile_pool(name="wpool", bufs=1))
        xpool = ctx.enter_context(tc.tile_pool(name="xpool", bufs=2))
        lgp = ctx.enter_context(tc.tile_pool(name="lgp", bufs=2, space="PSUM"))
        lgs = ctx.enter_context(tc.tile_pool(name="lgs", bufs=2))
        stp = ctx.enter_context(tc.tile_pool(name="stp", bufs=2, space="PSUM"))
        warmp = ctx.enter_context(tc.tile_pool(name="warmp", bufs=1, space="PSUM"))
        tk = ctx.enter_context(tc.tile_pool(name="tk", bufs=1))
        big = ctx.enter_context(tc.tile_pool(name="big", bufs=1))
        ohp = ctx.enter_context(tc.tile_pool(name="ohp", bufs=2))
        outp = ctx.enter_context(tc.tile_pool(name="outp", bufs=2))

        # ---- constants ----
        ident = consts.tile([P, P], f32)
        make_identity(nc, ident)

        bias_b = consts.tile([P, NEXP], f32)
        bias_ap = bass.AP(
            tensor=bias_d.tensor if hasattr(bias_d, "tensor") else bias_d,
            offset=0,
            ap=[[0, P], [1, NEXP]],
        )
        nc.gpsimd.dma_start(out=bias_b, in_=bias_ap)

        iota_u = consts.tile([P, NEXP], u32)
        nc.gpsimd.iota(iota_u, pattern=[[1, NEXP]], base=0, channel_multiplier=0)

        wt = wpool.tile([P, KT * NEXP], f32)
        nc.sync.dma_start(out=wt, in_=wt_d[:, :])
        w3 = wt.rearrange("p (k e) -> p k e", e=NEXP)


        chunks = CHUNKS if repeat == 1 else CHUNKS_LOOP

        def emit_body():
            scores_all = tk.tile([P, NTT * NEXP], f32, tag="scores_all",
                                 name="scores_all")
            for b in range(NB):
                lg = [lgp.tile([P, TB], f32, tag=f"lg{h}", name=f"lg{h}") for h in range(2)]
                k0 = 0
                for kc in chunks:
                    off = (b * KT + k0) * TB
                    xc = xpool.tile([P, max(chunks) * TB], f32, tag="xc")
                    nc.sync.dma_start(out=xc[:, 0:kc * TB],
                                      in_=xt_d[:, off:off + kc * TB])
                    for kk in range(kc):
                        k = k0 + kk
                        x_k = xc[:, kk * TB:(kk + 1) * TB]
                        for h in range(2):
                            w_k = w3[:, k, h * P:(h + 1) * P]
                            nc.tensor.matmul(lg[h], lhsT=w_k, rhs=x_k,
                                             start=(k == 0), stop=(k == KT - 1))
                    k0 += kc

                lgb = [lgs.tile([P, TB], f32, tag=f"lgs{h}", name=f"lgs{h}") for h in range(2)]
                for h in range(2):
                    nc.scalar.copy(out=lgb[h], in_=lg[h])
                for j in range(TB // P):
                    st = stp.tile([P, NEXP], f32)
                    for h in range(2):
                        nc.tensor.transpose(st[:, h * P:(h + 1) * P],
                                            lgb[h][:, j * P:(j + 1) * P], ident)
                    tt_i = b * (TB // P) + j
                    nc.scalar.activation(scores_all[:, tt_i * NEXP:(tt_i + 1) * NEXP],
                                         st, mybir.ActivationFunctionType.Sigmoid)

            # ---- batched noaux_tc top-8 across the 8 token tiles ----
            # Per-token-independent stages run as ONE wide DVE op over all
            # 8 tiles; only row-global sort ops stay per-tile.
            T = NTT
            G = N_GROUP
            EPG = NEXP // G

            sfc_all = big.tile([P, T * NEXP], f32, tag="sfc_all", name="sfc_all")
            nc.vector.tensor_tensor(
                sfc_all.rearrange("p (t e) -> p t e", e=NEXP),
                scores_all.rearrange("p (t e) -> p t e", e=NEXP),
                bias_b.rearrange("p (o e) -> p o e", o=1).to_broadcast([P, T, NEXP]),
                op=AO.add)
            g1_all = tk.tile([P, T * G], f32, tag="g1_all", name="g1_all")
            nc.vector.tensor_reduce(g1_all, sfc_all.rearrange("p (g e) -> p g e", e=EPG),
                                    axis=AX.X, op=AO.max)
            rep_all = big.tile([P, T * NEXP], f32, tag="rep_all", name="rep_all")
            for t in range(T):
                nc.vector.match_replace(
                    out=rep_all[:, t * NEXP:(t + 1) * NEXP],
                    in_to_replace=g1_all[:, t * G:(t + 1) * G],
                    in_values=sfc_all[:, t * NEXP:(t + 1) * NEXP],
                    imm_value=NEG_INF)
            gs_all = tk.tile([P, T * G], f32, tag="gs_all", name="gs_all")
            nc.vector.tensor_reduce(gs_all, rep_all.rearrange("p (g e) -> p g e", e=EPG),
                                    axis=AX.X, op=AO.max)
            nc.vector.tensor_add(gs_all, gs_all, g1_all)
            g8_all = tk.tile([P, T * 8], f32, tag="g8_all", name="g8_all")
            for t in range(T):
                nc.vector.max(out=g8_all[:, t * 8:(t + 1) * 8],
                              in_=gs_all[:, t * G:(t + 1) * G])
            # cmp = 1.0 where the group misses the top-TOPK_GROUP cut
            cmp_all = tk.tile([P, T * G], f32, tag="cmp_all", name="cmp_all")
            nc.vector.tensor_tensor(
                cmp_all.rearrange("p (t g) -> p t g", g=G),
                gs_all.rearrange("p (t g) -> p t g", g=G),
                g8_all.rearrange("p (t j) -> p t j", j=8)[:, :, TOPK_GROUP - 1:TOPK_GROUP]
                    .to_broadcast([P, T, G]),
                op=AO.is_lt)
            masked_all = big.tile([P, T * NEXP], f32, tag="masked_all", name="masked_all")
            nc.vector.scalar_tensor_tensor(
                masked_all.rearrange("p (g e) -> p g e", e=EPG),
                in0=cmp_all.rearrange("p (g o) -> p g o", o=1)
                    .to_broadcast([P, T * G, EPG]),
                scalar=NEG_INF,
                in1=sfc_all.rearrange("p (g e) -> p g e", e=EPG),
                op0=AO.mult, op1=AO.add)
            top8_all = tk.tile([P, T * 8], f32, tag="top8_all", name="top8_all")
            for t in range(T):
                nc.vector.max(out=top8_all[:, t * 8:(t + 1) * 8],
                              in_=masked_all[:, t * NEXP:(t + 1) * NEXP])
            us = outp.tile([P, T * 8], u32, tag="us", name="us")
            for t in range(T):
                nc.vector.max_index(us[:, t * 8:(t + 1) * 8],
                                    top8_all[:, t * 8:(t + 1) * 8],
                                    masked_all[:, t * NEXP:(t + 1) * NEXP])
            # idx leaves as soon as it exists; weights chain continues below
            nc.gpsimd.dma_start(out=oi_d.rearrange("(t p) j -> p t j", p=P),
                                in_=us.rearrange("p (t j) -> p t j", j=8))

            # gather unbiased scores at the selected indices (2 tiles/op)
            ssel_all = tk.tile([P, T * 8], f32, tag="ssel_all", name="ssel_all")
            for r in range(0, T, 2):
                oh = ohp.tile([P, 2 * 8 * NEXP], f32, tag="oh", name=f"oh{r}")
                nc.vector.tensor_tensor(
                    oh.rearrange("p (a e) -> p a e", e=NEXP),
                    us[:, r * 8:(r + 2) * 8].rearrange("p (a o) -> p a o", o=1)
                        .to_broadcast([P, 16, NEXP]),
                    iota_u.rearrange("p (o e) -> p o e", o=1)
                        .to_broadcast([P, 16, NEXP]),
                    op=AO.is_equal)
                nc.vector.tensor_tensor(
                    oh.rearrange("p (t a e) -> p t a e", t=2, e=NEXP),
                    oh.rearrange("p (t a e) -> p t a e", t=2, e=NEXP),
                    scores_all[:, r * NEXP:(r + 2) * NEXP]
                        .rearrange("p (t o e) -> p t o e", t=2, e=NEXP)
                        .to_broadcast([P, 2, 8, NEXP]),
                    op=AO.mult)
                nc.vector.tensor_reduce(
                    ssel_all[:, r * 8:(r + 2) * 8],
                    oh.rearrange("p (a e) -> p a e", e=NEXP),
                    axis=AX.X, op=AO.add)
            ssum_all = tk.tile([P, T], f32, tag="ssum_all", name="ssum_all")
            nc.vector.tensor_reduce(ssum_all, ssel_all.rearrange("p (t j) -> p t j", j=8),
                                    axis=AX.X, op=AO.add)
            rcp_all = tk.tile([P, T], f32, tag="rcp_all", name="rcp_all")
            nc.vector.reciprocal(rcp_all, ssum_all)
            wfin_all = outp.tile([P, T * 8], f32, tag="wfin_all", name="wfin_all")
            nc.vector.scalar_tensor_tensor(
                wfin_all.rearrange("p (t j) -> p t j", j=8),
                in0=ssel_all.rearrange("p (t j) -> p t j", j=8),
                scalar=ROUTED_SCALE,
                in1=rcp_all.rearrange("p (t o) -> p t o", o=1).to_broadcast([P, T, 8]),
                op0=AO.mult, op1=AO.mult)
            # the w store is the serial tail of the one-shot window and is
            # descriptor-bound (1024 x 32B runs); spread it over 4 DMA queues
            ow3 = ow_d.rearrange("(t p) j -> p t j", p=P)
            wf3 = wfin_all.rearrange("p (t j) -> p t j", j=8)
            for q, lo, hi in ((nc.sync, 0, 3), (nc.gpsimd, 3, 6),
                              (nc.scalar, 6, 8)):
                q.dma_start(out=ow3[:, lo:hi, :], in_=wf3[:, lo:hi, :])

        if repeat > 1:
            with tc.For_i(0, repeat, 1):
                emit_body()
        else:
            emit_body()

    if legalize:
        _legalize_waits(nc)
    return nc


_WAIT_SPLIT_SKIP = {"InstEventSemaphore", "InstUnconditionalBranch",
                    "InstCall", "InstRegisterMove", "InstConditionalBranch"}


def _legalize_waits(nc):
    """Walrus codegen allows a single sync-wait on most TPB instruction
    structs; hoist extra waits into standalone EventSemaphore instructions
    executed just before the offending instruction on the same engine."""
    import concourse.mybir as mybir

    for blk in nc.m.functions[0].blocks:
        out = []
        changed = False
        for inst in blk.instructions:
            si = getattr(inst, "sync_info", None)
            if (si is not None and len(si.on_wait) > 1
                    and type(inst).__name__ not in _WAIT_SPLIT_SKIP):
                waits = list(si.on_wait)
                for j, w in enumerate(waits[:-1]):
                    es = mybir.InstEventSemaphore(
                        name=f"{inst.name}-xw{j}", ins=[], outs=[])
                    es.engine = inst.engine
                    es.sync_info = mybir.SyncInfo(on_wait=[w], on_update=[])
                    out.append(es)
                inst.sync_info = mybir.SyncInfo(
                    on_wait=[waits[-1]], on_update=list(si.on_update))
                changed = True
            out.append(inst)
        if changed:
            blk.instructions = out


def _host_prep_w(weight):
    w32 = np.asarray(weight, dtype=np.float32)
    kt = w32.shape[1] // P
    # [e, (k p)] -> [p][k][e] pre-tiled so the device DMA is contiguous
    return np.ascontiguousarray(
        w32.T.reshape(kt, P, -1).transpose(1, 0, 2).reshape(P, -1))


def _host_prep_x(x):
    """x [TOKENS, HIDDEN] f32 -> per-core pre-tiled fp32:
    A[c][p, ((b*KT + k)*TB + t)] = x[c*T_CORE + b*TB + t, k*P + p]."""
    v = x.reshape(NCORES, NB, TB, KT, P)       # [c, b, t, k, p]
    v = v.transpose(0, 4, 1, 3, 2)             # [c, p, b, k, t]
    return np.ascontiguousarray(v).reshape(NCORES, P, NB * KT * TB)


_CACHED_NC = None


def kernel(hidden_states, weight, e_score_correction_bias):
    global _CACHED_NC
    from concourse.bass_utils import run_bass_kernel_spmd

    x = np.asarray(hidden_states, dtype=np.float32)
    b = np.asarray(e_score_correction_bias, dtype=np.float32)
    wt = _host_prep_w(weight)
    xt = _host_prep_x(x)

    if _CACHED_NC is None:
        _CACHED_NC = build_program()
    nc = _CACHED_NC

    in_maps = []
    for c in range(NCORES):
        in_maps.append({
            "xt": xt[c],
            "wt": wt,
            "bias": b,
        })
    res = run_bass_kernel_spmd(nc, in_maps, core_ids=list(range(NCORES)))
    idx = np.concatenate([r["topk_idx"] for r in res.results], axis=0)
    w = np.concatenate([r["topk_w"] for r in res.results], axis=0)
    return idx.astype(np.int64).astype(np.int32), w.astype(np.float32)
